# revision 8
# baseline (speedup 1.0000x reference)
"""Per-pixel depthwise 3x3 conv (Conv2dLocal) on 8 Trainium2 NeuronCores.

out[b,c,h,w] = sum_{i,j in 3x3} x[b,c,h+i-1,w+j-1] * weight[b, c*9+3i+j, h, w]

Sharding: 8 cores = 2 batches x 4 H-slabs of 64 rows (data/spatial parallel).
Host pads input spatially (1-px halo on H and W) and hands each core an
overlapping x slab, so the device program is identical and branch-free on
every core (pure SPMD, no collectives needed).

Per-core layout: partition p = hb*32 + c  (hb: 16-row block 0..3, c: channel
0..31); free dim = (row, w). 3x3 tap shifts are pure free-dim offsets.
Compute: DVE does the 9 elementwise multiplies; the TensorEngine accumulates
the 9 tap-product planes into PSUM via identity matmuls (psum += I @ prod);
ScalarE evacuates PSUM->SBUF; DMA streams weights in / results out.
"""

import sys

if "/opt/trn_rl_repo" not in sys.path:
    sys.path.insert(0, "/opt/trn_rl_repo")

from contextlib import ExitStack

import numpy as np

import concourse.bass as bass
import concourse.mybir as mybir
import concourse.tile as tile
from concourse import bacc
from concourse.bass_utils import run_bass_kernel_spmd
from concourse.masks import make_identity

# Problem shape (hardcoded per harness contract)
B, C, H, W = 2, 32, 256, 512
K = 3
KK = K * K
N_CORES = 8

# Per-core decomposition
HL = H // 4          # 64 local rows per core
HB = 4               # row-blocks per core (partition groups)
RB = HL // HB        # 16 rows per partition
G = 4                # rows processed per group (4 PSUM banks)
WP = W + 2           # width incl. halo
NP = 128             # partitions

FP32 = mybir.dt.float32

_PROGRAM = None


def _build_program() -> bass.Bass:
    nc = bacc.Bacc(
        "TRN2", target_bir_lowering=False, debug=False, num_devices=N_CORES
    )
    x_d = nc.declare_dram_parameter("x", [HB, C, RB + 2, WP], FP32, isOutput=False)
    w_d = nc.declare_dram_parameter("w", [C * KK, HL, W], FP32, isOutput=False)
    o_d = nc.declare_dram_parameter("o", [C, HL, W], FP32, isOutput=True)

    # weight view: [tap, hb, c, r, w]
    wv = w_d.rearrange("(c t) (hb r) w -> t hb c r w", t=KK, hb=HB)
    # output view: [hb, c, r, w] matching partition order p = hb*32 + c
    ov = o_d.rearrange("c (hb r) w -> hb c r w", hb=HB)

    with tile.TileContext(nc) as tc, ExitStack() as ctx:
        x_pool = ctx.enter_context(tc.tile_pool(name="x", bufs=1))
        const_pool = ctx.enter_context(tc.tile_pool(name="const", bufs=1))
        w_pool = ctx.enter_context(tc.tile_pool(name="wt", bufs=10))
        prod_pool = ctx.enter_context(tc.tile_pool(name="prod", bufs=3))
        out_pool = ctx.enter_context(tc.tile_pool(name="outsb", bufs=2))
        psum_pool = ctx.enter_context(tc.tile_pool(name="psum", bufs=2, space="PSUM"))

        ident = const_pool.tile([NP, NP], FP32)
        make_identity(nc, ident)

        # x slab: per partition 18 rows (16 + 2 halo) x 514 cols, loaded once
        x_sb = x_pool.tile([NP, RB + 2, WP], FP32)
        nc.sync.dma_start(out=x_sb[:], in_=x_d[:])

        for grp in range(RB // G):
            R = grp * G
            psum = psum_pool.tile([NP, G, W], FP32)
            for t in range(KK):
                i, j = t // K, t % K
                wt = w_pool.tile([NP, G, W], FP32, tag="wt")
                nc.sync.dma_start(out=wt, in_=wv[t, :, :, R : R + G, :])
                prod = prod_pool.tile([NP, G, W], FP32, tag="prod")
                nc.vector.tensor_tensor(
                    prod[:],
                    x_sb[:, R + i : R + i + G, j : j + W],
                    wt[:],
                    mybir.AluOpType.mult,
                )
                for s in range(G):
                    nc.tensor.matmul(
                        psum[:, s, :],
                        ident[:],
                        prod[:, s, :],
                        start=(t == 0),
                        stop=(t == KK - 1),
                    )
            out_sb = out_pool.tile([NP, G, W], FP32, tag="outsb")
            nc.scalar.copy(out=out_sb[:], in_=psum[:])
            nc.sync.dma_start(out=ov[:, :, R : R + G, :], in_=out_sb[:])

    nc.compile()
    return nc


def _get_program() -> bass.Bass:
    global _PROGRAM
    if _PROGRAM is None:
        _PROGRAM = _build_program()
    return _PROGRAM


def _shard_inputs(input: np.ndarray, weight: np.ndarray) -> list[dict]:
    xp = np.pad(input, ((0, 0), (0, 0), (1, 1), (1, 1)))
    in_maps = []
    for k in range(N_CORES):
        b, hb = k // 4, k % 4
        h0 = hb * HL
        xs = xp[b, :, h0 : h0 + HL + 2, :]  # [C, 66, WP]
        # expand into the HB overlapping 18-row windows: [HB, C, 18, WP]
        x4 = np.stack(
            [xs[:, r0 : r0 + RB + 2, :] for r0 in range(0, HL, RB)]
        ).astype(np.float32)
        ws = np.ascontiguousarray(weight[b, :, h0 : h0 + HL, :], dtype=np.float32)
        in_maps.append({"x": x4, "w": ws})
    return in_maps


def kernel(input: np.ndarray, weight: np.ndarray, _trace: bool = False):
    nc = _get_program()
    in_maps = _shard_inputs(np.asarray(input), np.asarray(weight))
    res = run_bass_kernel_spmd(
        nc, in_maps, core_ids=list(range(N_CORES)), trace=_trace
    )
    out = np.empty((B, C, H, W), dtype=np.float32)
    for k in range(N_CORES):
        b, hb = k // 4, k % 4
        out[b, :, hb * HL : (hb + 1) * HL, :] = res.results[k]["o"]
    if _trace:
        return out, res
    return out


# revision 10
# speedup vs baseline: 1.8862x; 1.8862x over previous
"""Per-pixel depthwise 3x3 conv (Conv2dLocal) on 8 Trainium2 NeuronCores.

out[b,c,h,w] = sum_{i,j in 3x3} x[b,c,h+i-1,w+j-1] * weight[b, c*9+3i+j, h, w]

Sharding: 8 cores = 2 batches x 4 H-slabs of 64 rows (data/spatial parallel).
The host pads the input spatially (1-px halo on H and W) and hands every core
an overlapping x slab, so the device program is identical and branch-free on
all cores (pure SPMD, no collectives).

Per-core layout: partition p = hb*32 + c (hb: 16-row block 0..3, c: channel);
free dim = (row, w), so all nine 3x3 tap shifts are free-dim offsets into a
single resident x slab [128, 18, 514].

DMA: the host pre-permutes the weight slab to [tap, grp, hb, c, r, w] so each
(tap, row-group) tile is one contiguous 1 MiB DRAM block — scattered-source
DMAs stripe over only 4 of the 16 SDMA engines (~110 GB/s), contiguous ones
use all 16 (~340 GB/s). Output uses a device-friendly contiguous layout too,
unscrambled on the host. Weights stream on the sync HWDGE ring; x and outputs
ride the scalar ring so they never queue behind the weight stream.

Compute: 9 DVE multiplies per group (fp32 tensor_tensor, 1x mode — uses only
DVE's dedicated SBUF port pair, so GpSimd runs truly in parallel) and the 8-way
tap accumulation split DVE/GpSimd. No PE/PSUM: fp32 matmul streams at 1/4 rate,
so identity-matmul accumulation is slower than DVE adds.
"""

import sys

if "/opt/trn_rl_repo" not in sys.path:
    sys.path.insert(0, "/opt/trn_rl_repo")

from contextlib import ExitStack

import numpy as np

import concourse.mybir as mybir
import concourse.tile as tile
from concourse import bacc
from concourse.bass_utils import run_bass_kernel_spmd

# Problem shape (hardcoded per harness contract)
B, C, H, W = 2, 32, 256, 512
K = 3
KK = K * K
N_CORES = 8

# Per-core decomposition
HL = H // 4          # 64 local rows per core
HB = 4               # row-blocks per core (partition groups)
RB = HL // HB        # 16 rows per partition
G = 4                # rows processed per group
NGRP = RB // G       # 4 groups
WP = W + 2           # width incl. halo
NP = 128             # partitions

FP32 = mybir.dt.float32
ADD = mybir.AluOpType.add
MULT = mybir.AluOpType.mult

_PROGRAM = None


def _build_program() -> bacc.Bacc:
    nc = bacc.Bacc(
        "TRN2", target_bir_lowering=False, debug=False, num_devices=N_CORES
    )
    x_d = nc.declare_dram_parameter("x", [HB, C, RB + 2, WP], FP32, isOutput=False)
    w_d = nc.declare_dram_parameter(
        "w", [KK, NGRP, HB, C, G, W], FP32, isOutput=False
    )
    o_d = nc.declare_dram_parameter("o", [NGRP, HB, C, G, W], FP32, isOutput=True)

    with tile.TileContext(nc) as tc, ExitStack() as ctx:
        x_pool = ctx.enter_context(tc.tile_pool(name="x", bufs=1))
        w_pool = ctx.enter_context(tc.tile_pool(name="wt", bufs=9))
        prod_pool = ctx.enter_context(tc.tile_pool(name="prod", bufs=4))
        acc_pool = ctx.enter_context(tc.tile_pool(name="acc", bufs=3))
        out_pool = ctx.enter_context(tc.tile_pool(name="outsb", bufs=2))

        # resident x slab: per partition 18 rows (16 + 2 halo) x 514 cols
        x_sb = x_pool.tile([NP, RB + 2, WP], FP32)
        nc.scalar.dma_start(out=x_sb[:], in_=x_d[:])

        for grp in range(NGRP):
            R = grp * G
            prods = []
            for t in range(KK):
                i, j = t // K, t % K
                wt = w_pool.tile([NP, G, W], FP32, tag="wt")
                nc.sync.dma_start(out=wt, in_=w_d[t, grp])
                prod = prod_pool.tile([NP, G, W], FP32, tag="prod")
                nc.vector.tensor_tensor(
                    prod[:],
                    x_sb[:, R + i : R + i + G, j : j + W],
                    wt[:],
                    MULT,
                )
                prods.append(prod)

            # GpSimd accumulates taps 0..5 (6 ops incl. the final combine),
            # DVE folds taps 6..8 (2 ops). fp32 TT never contends on SBUF
            # ports, so the two engines overlap fully.
            acc_p = acc_pool.tile([NP, G, W], FP32, tag="accp")
            nc.gpsimd.tensor_tensor(acc_p[:], prods[0][:], prods[1][:], ADD)
            for t in range(2, 6):
                nc.gpsimd.tensor_tensor(acc_p[:], acc_p[:], prods[t][:], ADD)
            acc_v = acc_pool.tile([NP, G, W], FP32, tag="accv")
            nc.vector.tensor_tensor(acc_v[:], prods[6][:], prods[7][:], ADD)
            nc.vector.tensor_tensor(acc_v[:], acc_v[:], prods[8][:], ADD)

            out_sb = out_pool.tile([NP, G, W], FP32, tag="outsb")
            nc.gpsimd.tensor_tensor(out_sb[:], acc_p[:], acc_v[:], ADD)
            nc.scalar.dma_start(out=o_d[grp], in_=out_sb[:])

    nc.compile()
    return nc


def _get_program() -> bacc.Bacc:
    global _PROGRAM
    if _PROGRAM is None:
        _PROGRAM = _build_program()
    return _PROGRAM


def _shard_inputs(input: np.ndarray, weight: np.ndarray) -> list[dict]:
    xp = np.pad(input, ((0, 0), (0, 0), (1, 1), (1, 1)))
    in_maps = []
    for k in range(N_CORES):
        b, hb = k // 4, k % 4
        h0 = hb * HL
        xs = xp[b, :, h0 : h0 + HL + 2, :]  # [C, 66, WP]
        # x: the HB overlapping 18-row windows -> [HB, C, 18, WP]
        x4 = np.stack(
            [xs[:, r0 : r0 + RB + 2, :] for r0 in range(0, HL, RB)]
        ).astype(np.float32)
        # weights: [C*KK, HL, W] -> [tap, grp, hb, c, r, w], contiguous per
        # (tap, grp) so each device DMA reads one linear 1 MiB block
        ws = (
            weight[b]
            .reshape(C, KK, H, W)[:, :, h0 : h0 + HL, :]
            .reshape(C, KK, HB, NGRP, G, W)
            .transpose(1, 3, 2, 0, 4, 5)
        )
        ws = np.ascontiguousarray(ws, dtype=np.float32)
        in_maps.append({"x": x4, "w": ws})
    return in_maps


def kernel(input: np.ndarray, weight: np.ndarray, _trace: bool = False):
    nc = _get_program()
    in_maps = _shard_inputs(np.asarray(input), np.asarray(weight))
    res = run_bass_kernel_spmd(
        nc, in_maps, core_ids=list(range(N_CORES)), trace=_trace
    )
    out = np.empty((B, C, H, W), dtype=np.float32)
    for k in range(N_CORES):
        b, hb = k // 4, k % 4
        # device out [grp, hb, c, r, w] -> [c, hb*16 + grp*4 + r, w]
        o = (
            res.results[k]["o"]
            .reshape(NGRP, HB, C, G, W)
            .transpose(2, 1, 0, 3, 4)
            .reshape(C, HL, W)
        )
        out[b, :, hb * HL : (hb + 1) * HL, :] = o
    if _trace:
        return out, res
    return out


# revision 11
# speedup vs baseline: 2.5315x; 1.3421x over previous
"""Per-pixel depthwise 3x3 conv (Conv2dLocal) on 8 Trainium2 NeuronCores.

out[b,c,h,w] = sum_{i,j in 3x3} x[b,c,h+i-1,w+j-1] * weight[b, c*9+3i+j, h, w]

Sharding: 8 cores = 2 batches x 4 H-slabs of 64 rows (data/spatial parallel).
The host pads the input spatially (1-px halo on H and W) and hands every core
an overlapping x slab, so the device program is identical and branch-free on
all cores (pure SPMD, no collectives).

Per-core layout: partition p = hb*32 + c (hb: 16-row block 0..3, c: channel);
free dim = (row, w), so all nine 3x3 tap shifts are free-dim offsets into a
single resident x slab [128, 18, 514].

DMA: the host pre-permutes the weight slab to [tap, grp, hb, c, r, w] so each
(tap, row-group) tile is one contiguous 1 MiB DRAM block — scattered-source
DMAs stripe over only 4 of the 16 SDMA engines (~110 GB/s), contiguous ones
use all 16 (~340 GB/s). Output uses a device-friendly contiguous layout too,
unscrambled on the host. Weights stream on the sync HWDGE ring; x and outputs
ride the scalar ring so they never queue behind the weight stream.

Compute: 9 DVE multiplies per group (fp32 tensor_tensor, 1x mode — uses only
DVE's dedicated SBUF port pair, so GpSimd runs truly in parallel) and the 8-way
tap accumulation split DVE/GpSimd. No PE/PSUM: fp32 matmul streams at 1/4 rate,
so identity-matmul accumulation is slower than DVE adds.
"""

import sys

if "/opt/trn_rl_repo" not in sys.path:
    sys.path.insert(0, "/opt/trn_rl_repo")

from contextlib import ExitStack

import numpy as np

import concourse.mybir as mybir
import concourse.tile as tile
from concourse import bacc
from concourse.bass_utils import run_bass_kernel_spmd

# Problem shape (hardcoded per harness contract)
B, C, H, W = 2, 32, 256, 512
K = 3
KK = K * K
N_CORES = 8

# Per-core decomposition
HL = H // 4          # 64 local rows per core
HB = 4               # row-blocks per core (partition groups)
RB = HL // HB        # 16 rows per partition
G = 4                # rows processed per group
NGRP = RB // G       # 4 groups
WP = W + 2           # width incl. halo
NP = 128             # partitions

FP32 = mybir.dt.float32
ADD = mybir.AluOpType.add
MULT = mybir.AluOpType.mult

_PROGRAM = None


def _build_program() -> bacc.Bacc:
    nc = bacc.Bacc(
        "TRN2", target_bir_lowering=False, debug=False, num_devices=N_CORES
    )
    x_d = nc.declare_dram_parameter("x", [HB, C, RB + 2, WP], FP32, isOutput=False)
    w_d = nc.declare_dram_parameter(
        "w", [KK, NGRP, HB, C, G, W], FP32, isOutput=False
    )
    o_d = nc.declare_dram_parameter("o", [NGRP, HB, C, G, W], FP32, isOutput=True)

    with tile.TileContext(nc) as tc, ExitStack() as ctx:
        x_pool = ctx.enter_context(tc.tile_pool(name="x", bufs=1))
        w_pool = ctx.enter_context(tc.tile_pool(name="wt", bufs=9))
        prod_pool = ctx.enter_context(tc.tile_pool(name="prod", bufs=4))
        acc_pool = ctx.enter_context(tc.tile_pool(name="acc", bufs=3))
        out_pool = ctx.enter_context(tc.tile_pool(name="outsb", bufs=2))
        xps_pool = ctx.enter_context(tc.tile_pool(name="xps", bufs=1, space="PSUM"))

        # resident x slab: per partition 18 rows (16 + 2 halo) x 514 cols
        x_sb = x_pool.tile([NP, RB + 2, WP], FP32)
        nc.scalar.dma_start(out=x_sb[:], in_=x_d[:])

        for grp in range(NGRP):
            R = grp * G
            # Stage this group's 6-row x window into PSUM via the idle
            # ScalarE, so the DVE multiplies read x through DVE's private
            # PSUM port + weights through its dedicated SBUF port — the
            # shared SBUF port pair stays free for GpSimd's adds.
            x_ps = xps_pool.tile([NP, G + 2, WP], FP32, tag="xps")
            nc.scalar.copy(out=x_ps[:], in_=x_sb[:, R : R + G + 2, :])
            prods = []
            for t in range(KK):
                i, j = t // K, t % K
                wt = w_pool.tile([NP, G, W], FP32, tag="wt")
                nc.sync.dma_start(out=wt, in_=w_d[t, grp])
                prod = prod_pool.tile([NP, G, W], FP32, tag="prod")
                nc.vector.tensor_tensor(
                    prod[:],
                    wt[:],
                    x_ps[:, i : i + G, j : j + W],
                    MULT,
                )
                prods.append(prod)

            # GpSimd accumulates taps 0..5 (6 ops incl. the final combine),
            # DVE folds taps 6..8 (2 ops). fp32 TT never contends on SBUF
            # ports, so the two engines overlap fully.
            acc_p = acc_pool.tile([NP, G, W], FP32, tag="accp")
            nc.gpsimd.tensor_tensor(acc_p[:], prods[0][:], prods[1][:], ADD)
            for t in range(2, 6):
                nc.gpsimd.tensor_tensor(acc_p[:], acc_p[:], prods[t][:], ADD)
            acc_v = acc_pool.tile([NP, G, W], FP32, tag="accv")
            nc.vector.tensor_tensor(acc_v[:], prods[6][:], prods[7][:], ADD)
            nc.vector.tensor_tensor(acc_v[:], acc_v[:], prods[8][:], ADD)

            out_sb = out_pool.tile([NP, G, W], FP32, tag="outsb")
            nc.gpsimd.tensor_tensor(out_sb[:], acc_p[:], acc_v[:], ADD)
            nc.scalar.dma_start(out=o_d[grp], in_=out_sb[:])

    nc.compile()
    return nc


def _get_program() -> bacc.Bacc:
    global _PROGRAM
    if _PROGRAM is None:
        _PROGRAM = _build_program()
    return _PROGRAM


def _shard_inputs(input: np.ndarray, weight: np.ndarray) -> list[dict]:
    xp = np.pad(input, ((0, 0), (0, 0), (1, 1), (1, 1)))
    in_maps = []
    for k in range(N_CORES):
        b, hb = k // 4, k % 4
        h0 = hb * HL
        xs = xp[b, :, h0 : h0 + HL + 2, :]  # [C, 66, WP]
        # x: the HB overlapping 18-row windows -> [HB, C, 18, WP]
        x4 = np.stack(
            [xs[:, r0 : r0 + RB + 2, :] for r0 in range(0, HL, RB)]
        ).astype(np.float32)
        # weights: [C*KK, HL, W] -> [tap, grp, hb, c, r, w], contiguous per
        # (tap, grp) so each device DMA reads one linear 1 MiB block
        ws = (
            weight[b]
            .reshape(C, KK, H, W)[:, :, h0 : h0 + HL, :]
            .reshape(C, KK, HB, NGRP, G, W)
            .transpose(1, 3, 2, 0, 4, 5)
        )
        ws = np.ascontiguousarray(ws, dtype=np.float32)
        in_maps.append({"x": x4, "w": ws})
    return in_maps


def kernel(input: np.ndarray, weight: np.ndarray, _trace: bool = False):
    nc = _get_program()
    in_maps = _shard_inputs(np.asarray(input), np.asarray(weight))
    res = run_bass_kernel_spmd(
        nc, in_maps, core_ids=list(range(N_CORES)), trace=_trace
    )
    out = np.empty((B, C, H, W), dtype=np.float32)
    for k in range(N_CORES):
        b, hb = k // 4, k % 4
        # device out [grp, hb, c, r, w] -> [c, hb*16 + grp*4 + r, w]
        o = (
            res.results[k]["o"]
            .reshape(NGRP, HB, C, G, W)
            .transpose(2, 1, 0, 3, 4)
            .reshape(C, HL, W)
        )
        out[b, :, hb * HL : (hb + 1) * HL, :] = o
    if _trace:
        return out, res
    return out
